# revision 5
# baseline (speedup 1.0000x reference)
"""MoE brute-force linear: o[t] = weight[gate[t]] @ inp[t].

Strategy: expert-parallel over 8 NeuronCores (2 experts/core).
  Host: stable-sort tokens by gate, pair the largest expert with the
  smallest (slot 0 / slot 1) on each core, pad each slot's token block to
  a uniform per-slot cap (multiple of 128), and pre-transpose activations
  and weights so the device kernel is pure GEMM with no on-chip
  transposes.
  Device: per expert, Y_e = X_e @ W_e^T as float32r (fp22) matmuls —
  full PE rate at N=512 — accumulating K=1024 over 8 PSUM passes.
  128-token stationary tiles, 512-wide moving weight tiles.
  DMA streams are decoupled: x loads on the SP HWDGE ring, weight loads
  on SWDGE (gpsimd), y stores on the ACT HWDGE ring.
"""

import numpy as np

BATCH = 8192
D = 1024
N_EXPERT = 16
N_CORES = 8
E_PER_CORE = N_EXPERT // N_CORES  # 2
KT = D // 128  # 8 contraction k-tiles
JC = D // 512  # 2 output column chunks

LAST_RESULT = None  # BassKernelResults of the most recent run


def _split_multiwait(nc):
    """Split every >1-sem-wait instruction into single-wait NoOps placed
    just before it on the same engine (this walrus rejects multi-wait
    CTRL instructions)."""
    import concourse.mybir as mybir

    for fn in nc.m.functions:
        for bb in fn.blocks:
            il = bb.instructions
            i = 0
            while i < len(il):
                ins = il[i]
                si = getattr(ins, "sync_info", None)
                if si is not None and len(si.on_wait) > 1:
                    waits = list(si.on_wait)
                    ins.sync_info = mybir.SyncInfo(
                        on_wait=[waits[-1]], on_update=list(si.on_update)
                    )
                    nops = [
                        mybir.InstNoOp(
                            name=f"{ins.name}-wsplit{k}",
                            engine=ins.engine,
                            sync_info=mybir.SyncInfo(on_wait=[w], on_update=[]),
                            bass_nofuse=True,
                        )
                        for k, w in enumerate(waits[:-1])
                    ]
                    il[i:i] = nops
                    i += len(nops)
                i += 1


def _build_program(Gs, reps=1):
    """Gs: per-slot group counts (tokens padded to Gs[i]*128 per slot)."""
    import concourse.bass as bass
    import concourse.tile as tile
    import concourse.mybir as mybir

    f32 = mybir.dt.float32
    f32r = mybir.dt.float32r

    nc = bass.Bass()
    # xT{i}[kp, g, kt, t] = X_e[g*128+t, kt*128+kp] -> per-group loads are
    # 4KB contiguous per partition line. f32r end-to-end: same 4-byte
    # storage, read by the PE at fp22 (full-rate matmul).
    xTs = [
        nc.dram_tensor(f"xT{i}", [128, Gs[i], KT, 128], f32r, kind="ExternalInput")
        for i in range(E_PER_CORE)
    ]
    # wT[i, kt, kp, j] = W_e[j, kt*128+kp]
    wT = nc.dram_tensor("wT", [E_PER_CORE, KT, 128, D], f32r, kind="ExternalInput")
    ys = [
        nc.dram_tensor(f"y{i}", [Gs[i] * 128, D], f32, kind="ExternalOutput")
        for i in range(E_PER_CORE)
    ]

    with tile.TileContext(nc) as tc:
        with (
            tc.tile_pool(name="wpool", bufs=2 * KT) as wpool,
            tc.tile_pool(name="xpool", bufs=6) as xpool,
            tc.tile_pool(name="opool", bufs=6) as opool,
            tc.tile_pool(name="pspool", bufs=8, space="PSUM") as pspool,
        ):
            for _ in range(reps):
                for i in range(E_PER_CORE):
                    wt = []
                    for kt in range(KT):
                        w_tile = wpool.tile([128, D], f32r, tag="w")
                        nc.gpsimd.dma_start(out=w_tile[:], in_=wT[i, kt])
                        wt.append(w_tile)
                    for g in range(Gs[i]):
                        xt = xpool.tile([128, KT, 128], f32r, tag="x")
                        nc.sync.dma_start(out=xt[:], in_=xTs[i][:, g])
                        ot = opool.tile([128, D], f32, tag="o")
                        for jc in range(JC):
                            ps = pspool.tile([128, 512], f32, tag="ps")
                            for kt in range(KT):
                                nc.tensor.matmul(
                                    ps[:],
                                    lhsT=xt[:, kt],
                                    rhs=wt[kt][:, jc * 512 : (jc + 1) * 512],
                                    start=(kt == 0),
                                    stop=(kt == KT - 1),
                                )
                            nc.vector.tensor_copy(
                                ot[:, jc * 512 : (jc + 1) * 512], ps[:]
                            )
                        nc.scalar.dma_start(
                            out=ys[i][g * 128 : (g + 1) * 128, :], in_=ot[:]
                        )
    _split_multiwait(nc)
    return nc


def _plan(counts):
    """Assign experts to (core, slot): slot 0 takes the 8 largest experts,
    slot 1 the 8 smallest, pairing rank c with rank 15-c for balance.
    Returns expert_of[core][slot] and per-slot group counts Gs."""
    rank = np.argsort(-counts, kind="stable")
    expert_of = [[int(rank[c]), int(rank[N_EXPERT - 1 - c])] for c in range(N_CORES)]
    Gs = []
    for i in range(E_PER_CORE):
        cap = max(int(counts[expert_of[c][i]]) for c in range(N_CORES))
        Gs.append(max(1, -(-cap // 128)))
    return expert_of, Gs


def _prep_inputs(inp, gate, weight):
    inp = np.ascontiguousarray(np.asarray(inp), dtype=np.float32)
    gate = np.asarray(gate).astype(np.int64)
    weight = np.ascontiguousarray(np.asarray(weight), dtype=np.float32)

    order = np.argsort(gate, kind="stable")
    counts = np.bincount(gate[order], minlength=N_EXPERT)
    starts = np.zeros(N_EXPERT + 1, dtype=np.int64)
    np.cumsum(counts, out=starts[1:])
    expert_of, Gs = _plan(counts)

    x_sorted = inp[order]  # [B, D]

    in_maps = []
    for c in range(N_CORES):
        m = {}
        wT = np.empty((E_PER_CORE, KT, 128, D), dtype=np.float32)
        for i in range(E_PER_CORE):
            e = expert_of[c][i]
            n_e = int(counts[e])
            P_i = Gs[i] * 128
            xe = np.zeros((P_i, D), dtype=np.float32)
            xe[:n_e] = x_sorted[starts[e] : starts[e] + n_e]
            # [P_i, D] -> [G, 128(t), KT, 128(kp)] -> [kp, g, kt, t]
            m[f"xT{i}"] = np.ascontiguousarray(
                xe.reshape(Gs[i], 128, KT, 128).transpose(3, 0, 2, 1)
            )
            # W_e [D_out, D_in] -> transpose -> [KT, 128(kp), D_out]
            wT[i] = weight[e].T.reshape(KT, 128, D)
        m["wT"] = wT
        in_maps.append(m)
    return in_maps, order, counts, starts, expert_of, Gs


def _gather_output(results, order, counts, starts, expert_of):
    out = np.empty((BATCH, D), dtype=np.float32)
    for c in range(N_CORES):
        for i in range(E_PER_CORE):
            e = expert_of[c][i]
            n_e = int(counts[e])
            if n_e:
                yc = results[c][f"y{i}"]
                out[order[starts[e] : starts[e] + n_e]] = yc[:n_e]
    return out


def kernel(inp, gate, weight):
    global LAST_RESULT
    from concourse.bass_utils import run_bass_kernel_spmd

    in_maps, order, counts, starts, expert_of, Gs = _prep_inputs(inp, gate, weight)
    nc = _build_program(Gs)

    last_err = None
    for attempt in range(3):
        try:
            res = run_bass_kernel_spmd(nc, in_maps, core_ids=list(range(N_CORES)))
            break
        except Exception as exc:  # transient NRT device errors: retry
            last_err = exc
    else:
        raise last_err
    LAST_RESULT = res

    return _gather_output(res.results, order, counts, starts, expert_of)


# revision 10
# speedup vs baseline: 1.3803x; 1.3803x over previous
"""MoE brute-force linear: o[t] = weight[gate[t]] @ inp[t].

Strategy: expert-parallel over 8 NeuronCores (2 experts/core).
  Host: stable-sort tokens by gate, pair the largest expert with the
  smallest (slot 0 / slot 1) on each core, pad each slot's token block to
  a uniform per-slot cap (multiple of 128), and pre-transpose activations
  and weights so the device kernel is pure GEMM with no on-chip
  transposes.
  Device: per expert, Y_e = X_e @ W_e^T as float32r (fp22) matmuls —
  full PE rate at N=512 — accumulating K=1024 over 8 PSUM passes.
  128-token stationary tiles, 512-wide moving weight tiles.
  DMA streams are decoupled: x loads on the SP HWDGE ring, weight loads
  on SWDGE (gpsimd), y stores on the ACT HWDGE ring.
"""

import numpy as np

BATCH = 8192
D = 1024
N_EXPERT = 16
N_CORES = 8
E_PER_CORE = N_EXPERT // N_CORES  # 2
KT = D // 128  # 8 contraction k-tiles
JC = D // 512  # 2 output column chunks

LAST_RESULT = None  # BassKernelResults of the most recent run


def _split_multiwait(nc):
    """Split every >1-sem-wait instruction into single-wait NoOps placed
    just before it on the same engine (this walrus rejects multi-wait
    CTRL instructions)."""
    import concourse.mybir as mybir

    for fn in nc.m.functions:
        for bb in fn.blocks:
            il = bb.instructions
            i = 0
            while i < len(il):
                ins = il[i]
                si = getattr(ins, "sync_info", None)
                if si is not None and len(si.on_wait) > 1:
                    waits = list(si.on_wait)
                    ins.sync_info = mybir.SyncInfo(
                        on_wait=[waits[-1]], on_update=list(si.on_update)
                    )
                    nops = [
                        mybir.InstNoOp(
                            name=f"{ins.name}-wsplit{k}",
                            engine=ins.engine,
                            sync_info=mybir.SyncInfo(on_wait=[w], on_update=[]),
                            bass_nofuse=True,
                        )
                        for k, w in enumerate(waits[:-1])
                    ]
                    il[i:i] = nops
                    i += len(nops)
                i += 1


def _build_program(Gs, Ts, reps=1):
    """Gs[i]: full 128-token groups per slot i; Ts[i]: tail-group token
    width (0 < Ts[i] <= 128, or 0 for no tail). Slot capacity is
    Gs[i]*128 + Ts[i] tokens."""
    import concourse.bass as bass
    import concourse.tile as tile
    import concourse.mybir as mybir

    f32 = mybir.dt.float32
    f32r = mybir.dt.float32r

    nc = bass.Bass()
    # xT{i}[kp, g, kt, t] = X_e[g*128+t, kt*128+kp] -> per-group loads are
    # 4KB contiguous per partition line. f32r end-to-end: same 4-byte
    # storage, read by the PE at fp22 (full-rate matmul).
    # wT[i, kt, kp, j] = W_e[j, kt*128+kp]
    wT = nc.dram_tensor("wT", [E_PER_CORE, KT, 128, D], f32r, kind="ExternalInput")
    xTs, xtails, ys, ytails = [], [], [], []
    for i in range(E_PER_CORE):
        xTs.append(
            nc.dram_tensor(f"xT{i}", [128, max(Gs[i], 1), KT, 128], f32r,
                           kind="ExternalInput")
            if Gs[i]
            else None
        )
        xtails.append(
            nc.dram_tensor(f"xtail{i}", [128, KT, Ts[i]], f32r, kind="ExternalInput")
            if Ts[i]
            else None
        )
        ys.append(
            nc.dram_tensor(f"y{i}", [Gs[i] * 128, D], f32, kind="ExternalOutput")
            if Gs[i]
            else None
        )
        ytails.append(
            nc.dram_tensor(f"ytail{i}", [Ts[i], D], f32, kind="ExternalOutput")
            if Ts[i]
            else None
        )

    with tile.TileContext(nc) as tc:
        with (
            tc.tile_pool(name="wpool", bufs=2 * KT) as wpool,
            tc.tile_pool(name="xpool", bufs=6) as xpool,
            tc.tile_pool(name="opool", bufs=6) as opool,
            tc.tile_pool(name="pspool", bufs=8, space="PSUM") as pspool,
        ):

            def do_group(wt, x_tile, y_ap, tw):
                # x_tile: [128(kp), KT, tw]; y_ap: [tw, D] in DRAM
                ot = opool.tile([128, D], f32, tag="o")
                for jc in range(JC):
                    ps = pspool.tile([128, 512], f32, tag="ps")
                    for kt in range(KT):
                        nc.tensor.matmul(
                            ps[:tw],
                            lhsT=x_tile[:, kt],
                            rhs=wt[kt][:, jc * 512 : (jc + 1) * 512],
                            start=(kt == 0),
                            stop=(kt == KT - 1),
                        )
                    nc.vector.tensor_copy(
                        ot[:tw, jc * 512 : (jc + 1) * 512], ps[:tw]
                    )
                nc.scalar.dma_start(out=y_ap, in_=ot[:tw])

            for _ in range(reps):
                for i in range(E_PER_CORE):
                    wt = []
                    for kt in range(KT):
                        w_tile = wpool.tile([128, D], f32r, tag="w")
                        nc.gpsimd.dma_start(out=w_tile[:], in_=wT[i, kt])
                        wt.append(w_tile)
                    for g in range(Gs[i]):
                        xt = xpool.tile([128, KT, 128], f32r, tag="x")
                        nc.sync.dma_start(out=xt[:], in_=xTs[i][:, g])
                        do_group(wt, xt, ys[i][g * 128 : (g + 1) * 128, :], 128)
                    if Ts[i]:
                        xt = xpool.tile([128, KT, Ts[i]], f32r, tag=f"xtail{i}")
                        nc.sync.dma_start(out=xt[:], in_=xtails[i][:])
                        do_group(wt, xt, ytails[i][:], Ts[i])
    _split_multiwait(nc)
    return nc


def _plan(counts):
    """Assign experts to (core, slot): slot 0 takes the 8 largest experts,
    slot 1 the 8 smallest, pairing rank c with rank 15-c for balance.
    Returns expert_of[core][slot], full-group counts Gs and tail widths Ts."""
    rank = np.argsort(-counts, kind="stable")
    expert_of = [[int(rank[c]), int(rank[N_EXPERT - 1 - c])] for c in range(N_CORES)]
    Gs, Ts = [], []
    for i in range(E_PER_CORE):
        cap = max(int(counts[expert_of[c][i]]) for c in range(N_CORES))
        cap = max(cap, 1)
        Gs.append(cap // 128)
        Ts.append(cap - (cap // 128) * 128)
    return expert_of, Gs, Ts


def _prep_inputs(inp, gate, weight):
    inp = np.ascontiguousarray(np.asarray(inp), dtype=np.float32)
    gate = np.asarray(gate).astype(np.int64)
    weight = np.ascontiguousarray(np.asarray(weight), dtype=np.float32)

    order = np.argsort(gate, kind="stable")
    counts = np.bincount(gate[order], minlength=N_EXPERT)
    starts = np.zeros(N_EXPERT + 1, dtype=np.int64)
    np.cumsum(counts, out=starts[1:])
    expert_of, Gs, Ts = _plan(counts)

    x_sorted = inp[order]  # [B, D]

    in_maps = []
    for c in range(N_CORES):
        m = {}
        wT = np.empty((E_PER_CORE, KT, 128, D), dtype=np.float32)
        for i in range(E_PER_CORE):
            e = expert_of[c][i]
            n_e = int(counts[e])
            P_i = Gs[i] * 128 + Ts[i]
            xe = np.zeros((P_i, D), dtype=np.float32)
            xe[:n_e] = x_sorted[starts[e] : starts[e] + n_e]
            if Gs[i]:
                # [G*128, D] -> [G, 128(t), KT, 128(kp)] -> [kp, g, kt, t]
                m[f"xT{i}"] = np.ascontiguousarray(
                    xe[: Gs[i] * 128]
                    .reshape(Gs[i], 128, KT, 128)
                    .transpose(3, 0, 2, 1)
                )
            if Ts[i]:
                # [T, D] -> [T(t), KT, 128(kp)] -> [kp, kt, t]
                m[f"xtail{i}"] = np.ascontiguousarray(
                    xe[Gs[i] * 128 :].reshape(Ts[i], KT, 128).transpose(2, 1, 0)
                )
            # W_e [D_out, D_in] -> transpose -> [KT, 128(kp), D_out]
            wT[i] = weight[e].T.reshape(KT, 128, D)
        m["wT"] = wT
        in_maps.append(m)
    return in_maps, order, counts, starts, expert_of, Gs, Ts


def _gather_output(results, order, counts, starts, expert_of, Gs, Ts):
    out = np.empty((BATCH, D), dtype=np.float32)
    for c in range(N_CORES):
        for i in range(E_PER_CORE):
            e = expert_of[c][i]
            n_e = int(counts[e])
            if not n_e:
                continue
            full = Gs[i] * 128
            n_full = min(n_e, full)
            idx = order[starts[e] : starts[e] + n_e]
            if n_full:
                out[idx[:n_full]] = results[c][f"y{i}"][:n_full]
            if n_e > full:
                out[idx[full:]] = results[c][f"ytail{i}"][: n_e - full]
    return out


def kernel(inp, gate, weight):
    global LAST_RESULT
    from concourse.bass_utils import run_bass_kernel_spmd

    in_maps, order, counts, starts, expert_of, Gs, Ts = _prep_inputs(
        inp, gate, weight
    )
    nc = _build_program(Gs, Ts)

    last_err = None
    for attempt in range(3):
        try:
            res = run_bass_kernel_spmd(nc, in_maps, core_ids=list(range(N_CORES)))
            break
        except Exception as exc:  # transient NRT device errors: retry
            last_err = exc
            import time

            time.sleep(2.0 * (attempt + 1))
    else:
        raise last_err
    LAST_RESULT = res

    return _gather_output(res.results, order, counts, starts, expert_of, Gs, Ts)
